# revision 22
# baseline (speedup 1.0000x reference)
"""Trainium2 Bass kernel for nn_Attention_83502754169400.

16-head causal attention (B=2, S=2048, D=2048) with qk-LayerNorm + RoPE,
tensor-parallel over heads on 8 NeuronCores (2 heads/core), AllToAll to
re-shard by sequence rows before a row-sharded wo matmul, so the final
output is a pure concat (no all-reduce).

Per-core pipeline:
  1. QKV projection (x^T streamed as lhsT, per-core [wq|wk|wv] as rhs, bf16),
     LN stats + apply + RoPE in groups of 8 row-tiles so the vector engines
     trail the projection matmuls instead of serializing after them.
  2. Causal flash-style attention per (head, batch): scores -> exp (scale
     folded into the activation) -> denominator scale -> xbar-DMA transpose
     -> per-row-block PV with V stationary (gives av^T).
  3. AllToAll split in two (one per local head): the first overlaps the
     second head's attention, the second overlaps the even-head wo matmuls.
  4. wo matmul over all 16 heads for this core's 512 rows -> fp32 out slice.
"""

import math
import sys

import numpy as np

sys.path.insert(0, "/opt/trn_rl_repo")

# Problem constants (hardcoded per contract).
B, S, D, H = 2, 2048, 2048, 16
HD = 128
NCORES = 8
HL = H // NCORES      # heads per core = 2
EPS = 1e-5
SCALE = 1.0 / math.sqrt(HD)
P = 128
ST = 32               # (b, s) row tiles of 128
DC = 16               # d_model chunks of 128
BS = B * S            # 4096

_NC_CACHE = {}


def build_nc():
    if "nc" in _NC_CACHE:
        return _NC_CACHE["nc"]

    import concourse.mybir as mybir
    from concourse import bacc
    from concourse.bass import ts
    from concourse.tile import TileContext
    from concourse.tile_rust import add_dep_helper

    bf16 = mybir.dt.bfloat16
    f32 = mybir.dt.float32
    AF = mybir.ActivationFunctionType
    OP = mybir.AluOpType

    nc = bacc.Bacc()

    xt = nc.declare_dram_parameter("xt", [DC, P, BS], bf16, isOutput=False)
    w3 = nc.declare_dram_parameter("w3", [DC, P, 768], bf16, isOutput=False)
    wo = nc.declare_dram_parameter("wo", [H, P, D], bf16, isOutput=False)
    fc = nc.declare_dram_parameter("fc", [16, P, 64], bf16, isOutput=False)
    fs = nc.declare_dram_parameter("fs", [16, P, 64], bf16, isOutput=False)
    out = nc.declare_dram_parameter("out", [4, P, D], f32, isOutput=True)

    # one AllToAll per local head; block r = av^T for dest core r's 512 rows
    a2a_in = [nc.dram_tensor(f"a2a_in{i}", [NCORES, P, 512], bf16)
              for i in range(HL)]
    a2a_out = [nc.dram_tensor(f"a2a_out{i}", [NCORES, P, 512], bf16)
               for i in range(HL)]

    with TileContext(nc) as tc:
        with (
            tc.tile_pool(name="persist", bufs=1) as persist,
            tc.tile_pool(name="small", bufs=4) as small,
        ):
            # ---- persistent SBUF tensors --------------------------------
            # split per batch / per 8-tile group so Tile's per-tile dep
            # tracking doesn't serialize consumers on unrelated producers
            v_b = [persist.tile([P, 16, HL * HD], bf16, name=f"v_b{i}")
                   for i in range(B)]
            qk_b = [persist.tile([P, 4, 16, P], bf16, name=f"qk_b{i}")
                    for i in range(B)]
            dmask = persist.tile([P, P], f32)                # causal additive
            av_blk = [persist.tile([P, NCORES, 512], bf16, name=f"av_blk{i}")
                      for i in range(HL)]

            nc.gpsimd.memset(dmask, 0.0)
            nc.gpsimd.affine_select(
                out=dmask, in_=dmask, compare_op=OP.is_ge,
                fill=-1e9, base=0, pattern=[[-1, P]], channel_multiplier=1,
            )

            # ---- stage 1: QKV projection + LN + RoPE, groups of 8 -------
            with (
                tc.tile_pool(name="s1", bufs=1) as s1,
                tc.tile_pool(name="s1w", bufs=6) as s1w,
                tc.tile_pool(name="s1x", bufs=4) as s1x,
                tc.tile_pool(name="ps_qkv", bufs=3, space="PSUM") as ps_qkv,
            ):
                w3_sb = s1.tile([P, DC, 768], bf16)
                nc.sync.dma_start(w3_sb, w3[:, :, :].rearrange("a p c -> p a c"))
                fc_sb = s1.tile([P, 16, 64], bf16)
                fs_sb = s1.tile([P, 16, 64], bf16)
                nc.sync.dma_start(fc_sb, fc[:, :, :].rearrange("a p c -> p a c"))
                nc.sync.dma_start(fs_sb, fs[:, :, :].rearrange("a p c -> p a c"))
                for st in range(ST):
                    sg, r4 = st // 4, st % 4
                    if r4 == 0:
                        xt_g = s1x.tile([P, DC, 512], bf16, tag="xt",
                                        name=f"xt_{sg}")
                        nc.gpsimd.dma_start(
                            xt_g,
                            xt[:, :, ts(sg, 512)].rearrange("a p c -> p a c"))
                    ps = ps_qkv.tile([P, 768], f32, tag="psqkv",
                                     name=f"psqkv_{st}")
                    for dc in range(DC):
                        nc.tensor.matmul(
                            ps[:, 0:512], lhsT=xt_g[:, dc, ts(r4, P)],
                            rhs=w3_sb[:, dc, 0:512],
                            start=(dc == 0), stop=(dc == DC - 1))
                        nc.tensor.matmul(
                            ps[:, 512:768], lhsT=xt_g[:, dc, ts(r4, P)],
                            rhs=w3_sb[:, dc, 512:768],
                            start=(dc == 0), stop=(dc == DC - 1))
                    qkr = s1w.tile([P, 512], bf16, tag="qkr",
                                   name=f"qkr_{st}")
                    nc.scalar.copy(qkr, ps[:, 0:512])
                    nc.scalar.copy(v_b[st // 16][:, st % 16, :],
                                   ps[:, 512:768])
                    # LN stats (squares on DVE so ACT keeps one table set)
                    sq = s1w.tile([P, 512], bf16, tag="sq", name=f"sq_{st}")
                    nc.vector.tensor_tensor(sq, qkr, qkr, OP.mult)
                    st8 = s1w.tile([P, 8], f32, tag="st8", name=f"st8_{st}")
                    nc.vector.reduce_sum(
                        st8[:, 0:4],
                        ps[:, 0:512].rearrange("p (c x) -> p c x", c=4),
                        axis=mybir.AxisListType.X)
                    nc.vector.reduce_sum(
                        st8[:, 4:8],
                        sq.rearrange("p (c x) -> p c x", c=4),
                        axis=mybir.AxisListType.X)
                    mu4 = s1w.tile([P, 4], f32, tag="mu4", name=f"mu4_{st}")
                    var4 = s1w.tile([P, 4], f32, tag="var4", name=f"var4_{st}")
                    std4 = s1w.tile([P, 4], f32, tag="std4", name=f"std4_{st}")
                    rstd4 = s1w.tile([P, 4], f32, tag="rstd4",
                                     name=f"rstd4_{st}")
                    nmr4 = s1w.tile([P, 4], f32, tag="nmr4", name=f"nmr4_{st}")
                    nc.vector.tensor_scalar_mul(mu4, st8[:, 0:4], 1.0 / HD)
                    nc.vector.tensor_scalar(
                        var4, st8[:, 4:8], 1.0 / HD, EPS, OP.mult, OP.add)
                    nc.vector.tensor_tensor(std4, mu4, mu4, OP.mult)
                    nc.vector.tensor_tensor(var4, var4, std4, OP.subtract)
                    nc.scalar.sqrt(std4, var4)
                    nc.vector.reciprocal(rstd4, std4)
                    nc.vector.tensor_tensor(nmr4, mu4, rstd4, OP.mult)
                    nc.vector.tensor_scalar_mul(nmr4, nmr4, -1.0)
                    # LN apply + RoPE
                    sidx = st % 16
                    qkd = qk_b[st // 16]
                    for c4 in range(4):
                        nc.scalar.activation(
                            qkd[:, c4, sidx, :], qkr[:, ts(c4, P)],
                            AF.Identity,
                            bias=nmr4[:, c4:c4 + 1],
                            scale=rstd4[:, c4:c4 + 1])
                    cb = fc_sb[:, sidx:sidx + 1, :].to_broadcast((P, 4, 64))
                    sb_ = fs_sb[:, sidx:sidx + 1, :].to_broadcast((P, 4, 64))
                    e = qkd[:, :, sidx, 0:64]
                    o = qkd[:, :, sidx, 64:128]
                    t1 = s1w.tile([P, 4, 64], bf16, tag="t1")
                    t2 = s1w.tile([P, 4, 64], bf16, tag="t2")
                    t3 = s1w.tile([P, 4, 64], bf16, tag="t3")
                    t4 = s1w.tile([P, 4, 64], bf16, tag="t4")
                    nc.vector.tensor_tensor(t1, e, cb, OP.mult)
                    nc.vector.tensor_tensor(t2, o, sb_, OP.mult)
                    nc.vector.tensor_tensor(t3, e, sb_, OP.mult)
                    nc.vector.tensor_tensor(t4, o, cb, OP.mult)
                    nc.vector.tensor_tensor(e, t1, t2, OP.subtract)
                    nc.vector.tensor_tensor(o, t3, t4, OP.add)

            # ---- attention --------------------------------------------
            # software-pipelined per 4-row group: scores/exp/scale/transpose
            # for group g+1 are issued before PV of group g, so the PE has a
            # full group of runway while transposes complete.
            a2a_writes = [[], []]
            with (
                tc.tile_pool(name="att", bufs=2) as att,
                tc.tile_pool(name="attp", bufs=3) as attp,
                tc.tile_pool(name="attw", bufs=6) as attw,
                tc.tile_pool(name="ps_s", bufs=6, space="PSUM") as ps_s,
                tc.tile_pool(name="ps_av", bufs=2, space="PSUM") as ps_av,
            ):
                for hl in range(HL):
                    for b in range(B):
                        qt = att.tile([P, 16, P], bf16, tag="qt")
                        kt = att.tile([P, 16, P], bf16, tag="kt")
                        nc.sync.dma_start_transpose(
                            qt, qk_b[b][:, hl, :, :])
                        nc.sync.dma_start_transpose(
                            kt, qk_b[b][:, 2 + hl, :, :])

                        pts = {}

                        def scores_phase(g, qt=qt, kt=kt, pts=pts,
                                         hl=hl, b=b):
                            pt = attp.tile([P, 16, 512], bf16, tag="pt",
                                           name=f"pt_{hl}_{b}_{g}")
                            pts[g] = pt
                            for r in range(4):
                                qi = 4 * g + r
                                w_row = (qi + 1) * P
                                prow = attw.tile([P, S], bf16, tag="prow",
                                                 name=f"prow_{r}")
                                nfull = (qi // 4) * 4
                                for c0 in range(0, nfull, 4):
                                    pss = ps_s.tile([P, 512], f32, tag="pss")
                                    nc.tensor.matmul(
                                        pss, lhsT=qt[:, qi, :],
                                        rhs=kt[:, c0:c0 + 4, :],
                                        start=True, stop=True)
                                    nc.scalar.activation(
                                        prow[:, c0 * P:(c0 + 4) * P], pss,
                                        AF.Exp, scale=SCALE)
                                rw = (qi - nfull + 1) * P
                                pss = ps_s.tile([P, 512], f32, tag="pss")
                                nc.tensor.matmul(
                                    pss[:, 0:rw], lhsT=qt[:, qi, :],
                                    rhs=kt[:, nfull:qi + 1, :],
                                    start=True, stop=True)
                                nc.vector.tensor_tensor(
                                    pss[:, rw - P:rw], pss[:, rw - P:rw],
                                    dmask, OP.add)
                                nc.scalar.activation(
                                    prow[:, nfull * P:nfull * P + rw],
                                    pss[:, 0:rw], AF.Exp, scale=SCALE)
                                den = small.tile([P, 1], f32, tag="den")
                                rden = small.tile([P, 1], f32, tag="rden")
                                nc.vector.reduce_sum(
                                    den, prow[:, 0:w_row],
                                    axis=mybir.AxisListType.X)
                                nc.vector.reciprocal(rden, den)
                                prs = attw.tile([P, S], bf16, tag="prs",
                                                name=f"prs_{r}")
                                nc.vector.tensor_scalar_mul(
                                    prs[:, 0:w_row], prow[:, 0:w_row], rden)
                                nc.sync.dma_start_transpose(
                                    pt[:, 0:qi + 1, r * P:(r + 1) * P],
                                    prs[:, 0:w_row])

                        def pv_phase(g, pts=pts, hl=hl, b=b):
                            pt = pts.pop(g)
                            av = ps_av.tile([P, 512], f32, tag="av",
                                            name=f"av_{hl}_{b}_{g}")
                            for r in range(4):
                                qi = 4 * g + r
                                for kj in range(qi + 1):
                                    nc.tensor.matmul(
                                        av[:, r * P:(r + 1) * P],
                                        lhsT=v_b[b][:, kj,
                                                    hl * HD:(hl + 1) * HD],
                                        rhs=pt[:, kj, r * P:(r + 1) * P],
                                        start=(kj == 0), stop=(kj == qi))
                                nc.vector.tensor_copy(
                                    av_blk[hl][:, 4 * b + g,
                                               r * P:(r + 1) * P],
                                    av[:, r * P:(r + 1) * P])

                        for g in range(6):
                            if g < 4:
                                scores_phase(g)
                            if g >= 2:
                                pv_phase(g - 2)

                    for j in range(NCORES):
                        wr = nc.gpsimd.dma_start(a2a_in[hl][j],
                                                 av_blk[hl][:, j, :])
                        a2a_writes[hl].append(wr)
            # ---- AllToAll x2 (second overlaps even-head wo matmuls) ----
            colls = []
            for hl in range(HL):
                coll = nc.gpsimd.collective_compute(
                    "AllToAll", OP.bypass,
                    ins=[a2a_in[hl][:, :, :]],
                    outs=[a2a_out[hl][:, :, :]],
                    replica_groups=[list(range(NCORES))],
                )
                for wr in a2a_writes[hl]:
                    add_dep_helper(coll.ins, wr.ins,
                                   reason="a2a needs staged inputs")
                colls.append(coll)

            # ---- wo: out rows = this core's 512, all 16 heads ----------
            # head h lives in a2a_out[h % 2], block h // 2; process heads of
            # a2a#0 first so they overlap a2a#1's wire time.
            with (
                tc.tile_pool(name="wop", bufs=1) as wop,
                tc.tile_pool(name="wopf", bufs=10) as wopf,
                tc.tile_pool(name="wow", bufs=3) as wow,
                tc.tile_pool(name="ps_o", bufs=2, space="PSUM") as ps_o,
            ):
                avr = [wop.tile([P, NCORES, 512], bf16, name=f"avr{i}")
                       for i in range(HL)]
                for i in range(HL):
                    for src in range(NCORES):
                        rd = nc.scalar.dma_start(avr[i][:, src, :],
                                                 a2a_out[i][src])
                        add_dep_helper(rd.ins, colls[i].ins,
                                       reason="read a2a output")
                horder = [2 * c for c in range(NCORES)] + \
                         [2 * c + 1 for c in range(NCORES)]
                for nchk in range(4):
                    pss4 = [ps_o.tile([P, 512], f32, tag=f"po{st4}",
                                      name=f"po_{nchk}_{st4}")
                            for st4 in range(4)]
                    for ih, h in enumerate(horder):
                        wot = wopf.tile([P, 512], bf16, tag="wot",
                                        name=f"wot_{nchk}_{h}")
                        nc.gpsimd.dma_start(wot, wo[h, :, ts(nchk, 512)])
                        for st4 in range(4):
                            nc.tensor.matmul(
                                pss4[st4],
                                lhsT=avr[h % 2][:, h // 2, ts(st4, P)],
                                rhs=wot, start=(ih == 0), stop=(ih == H - 1))
                    for st4 in range(4):
                        ot = wow.tile([P, 512], f32, tag="ot")
                        nc.vector.tensor_copy(ot, pss4[st4])
                        nc.sync.dma_start(out[st4, :, ts(nchk, 512)], ot)

    nc.compile()
    _NC_CACHE["nc"] = nc
    return nc


def _perm_even_odd():
    # column permutation de-interleaving RoPE pairs: [0,2,...,126,1,3,...,127]
    return np.concatenate([np.arange(0, HD, 2), np.arange(1, HD, 2)])


def prep_in_maps(inputs):
    import ml_dtypes

    bf = ml_dtypes.bfloat16
    x = np.asarray(inputs["x"], dtype=np.float32).reshape(BS, D)
    wq = np.asarray(inputs["wq"], dtype=np.float32)
    wk = np.asarray(inputs["wk"], dtype=np.float32)
    wv = np.asarray(inputs["wv"], dtype=np.float32)
    wo = np.asarray(inputs["wo"], dtype=np.float32)
    fc = np.asarray(inputs["freqs_cos"], dtype=np.float32)
    fs = np.asarray(inputs["freqs_sin"], dtype=np.float32)

    xt = np.ascontiguousarray(x.T).astype(bf).reshape(DC, P, BS)
    wo_b = np.ascontiguousarray(wo).astype(bf).reshape(H, P, D)
    fc_b = np.ascontiguousarray(fc.reshape(16, P, 64)).astype(bf)
    fs_b = np.ascontiguousarray(fs.reshape(16, P, 64)).astype(bf)

    perm = _perm_even_odd()
    in_maps = []
    for c in range(NCORES):
        cols = []
        for w in (wq, wk):
            for hl in range(HL):
                h = HL * c + hl
                cols.append(w[:, h * HD:(h + 1) * HD][:, perm])
        cols.append(wv[:, HL * c * HD:(HL * c + HL) * HD])
        w3_c = np.concatenate(cols, axis=1).astype(bf).reshape(DC, P, 768)
        in_maps.append({
            "xt": xt, "w3": np.ascontiguousarray(w3_c), "wo": wo_b,
            "fc": fc_b, "fs": fs_b,
        })
    return in_maps


def assemble(results):
    parts = [np.asarray(r["out"], dtype=np.float32).reshape(512, D)
             for r in results]
    return np.concatenate(parts, axis=0).reshape(B, S, D)


def _is_supported(inputs):
    mask = np.asarray(inputs["mask"], dtype=np.float32)
    if mask.shape != (S, S):
        return False
    tril = np.tril(np.ones((S, S), dtype=bool))
    if not (np.all(mask[tril] == 0.0) and np.all(mask[~tril] <= -1e8)):
        return False
    for k, v in (("q_ln_w", 1.0), ("q_ln_b", 0.0),
                 ("k_ln_w", 1.0), ("k_ln_b", 0.0)):
        if not np.allclose(np.asarray(inputs[k], dtype=np.float32), v):
            return False
    return True


def _kernel_numpy(inputs):
    x = np.asarray(inputs["x"], dtype=np.float32)
    fc, fs = np.asarray(inputs["freqs_cos"]), np.asarray(inputs["freqs_sin"])
    mask = np.asarray(inputs["mask"])
    wq, wk, wv, wo = (np.asarray(inputs[k]) for k in ("wq", "wk", "wv", "wo"))
    qw, qb = np.asarray(inputs["q_ln_w"]), np.asarray(inputs["q_ln_b"])
    kw, kb = np.asarray(inputs["k_ln_w"]), np.asarray(inputs["k_ln_b"])

    def ln(t, w, b):
        mu = t.mean(-1, keepdims=True)
        var = ((t - mu) ** 2).mean(-1, keepdims=True)
        return (t - mu) / np.sqrt(var + EPS) * w + b

    def rope(t):
        e, o = t[..., 0::2], t[..., 1::2]
        c = fc[None, :, None, :]
        s = fs[None, :, None, :]
        outp = np.empty_like(t)
        outp[..., 0::2] = e * c - o * s
        outp[..., 1::2] = e * s + o * c
        return outp

    b, s, _ = x.shape
    q = rope(ln((x @ wq).reshape(b, s, H, HD), qw, qb))
    k = rope(ln((x @ wk).reshape(b, s, H, HD), kw, kb))
    v = (x @ wv).reshape(b, s, H, HD)
    outp = np.empty((b, s, H, HD), dtype=np.float32)
    for bi in range(b):
        for h in range(H):
            sc = (q[bi, :, h, :] @ k[bi, :, h, :].T) * SCALE + mask
            sc -= sc.max(-1, keepdims=True)
            p = np.exp(sc)
            p /= p.sum(-1, keepdims=True)
            outp[bi, :, h, :] = p @ v[bi, :, h, :]
    return (outp.reshape(b, s, D) @ wo).astype(np.float32)


def kernel(**inputs) -> np.ndarray:
    if not _is_supported(inputs):
        return _kernel_numpy(inputs)
    from concourse.bass_utils import run_bass_kernel_spmd

    nc = build_nc()
    in_maps = prep_in_maps(inputs)
    res = run_bass_kernel_spmd(nc, in_maps, list(range(NCORES)))
    return assemble(res.results)


if __name__ == "__main__":
    import reference

    inputs = {k: np.asarray(v) for k, v in reference.setup_inputs().items()}
    expected = np.asarray(reference.reference(**inputs))
    actual = kernel(**inputs)
    rel = np.abs(actual - expected).max() / (np.abs(expected).max() + 1e-30)
    print(f"Relative error: {rel:.3e}")


# revision 23
# speedup vs baseline: 1.0496x; 1.0496x over previous
"""Trainium2 Bass kernel for nn_Attention_83502754169400.

16-head causal attention (B=2, S=2048, D=2048) with qk-LayerNorm + RoPE,
tensor-parallel over heads on 8 NeuronCores (2 heads/core), AllToAll to
re-shard by sequence rows before a row-sharded wo matmul, so the final
output is a pure concat (no all-reduce).

Per-core pipeline:
  1. QKV projection (x^T streamed as lhsT, per-core [wq|wk|wv] as rhs, bf16),
     LN stats + apply + RoPE in groups of 8 row-tiles so the vector engines
     trail the projection matmuls instead of serializing after them.
  2. Causal flash-style attention per (head, batch): scores -> exp (scale
     folded into the activation) -> denominator scale -> xbar-DMA transpose
     -> per-row-block PV with V stationary (gives av^T).
  3. AllToAll split in two (one per local head): the first overlaps the
     second head's attention, the second overlaps the even-head wo matmuls.
  4. wo matmul over all 16 heads for this core's 512 rows -> fp32 out slice.
"""

import math
import sys

import numpy as np

sys.path.insert(0, "/opt/trn_rl_repo")

# Problem constants (hardcoded per contract).
B, S, D, H = 2, 2048, 2048, 16
HD = 128
NCORES = 8
HL = H // NCORES      # heads per core = 2
EPS = 1e-5
SCALE = 1.0 / math.sqrt(HD)
P = 128
ST = 32               # (b, s) row tiles of 128
DC = 16               # d_model chunks of 128
BS = B * S            # 4096

_NC_CACHE = {}


def build_nc():
    if "nc" in _NC_CACHE:
        return _NC_CACHE["nc"]

    import concourse.mybir as mybir
    from concourse import bacc
    from concourse.bass import ts
    from concourse.tile import TileContext
    from concourse.tile_rust import add_dep_helper

    bf16 = mybir.dt.bfloat16
    f32 = mybir.dt.float32
    AF = mybir.ActivationFunctionType
    OP = mybir.AluOpType

    nc = bacc.Bacc()

    xt = nc.declare_dram_parameter("xt", [DC, P, BS], bf16, isOutput=False)
    w3 = nc.declare_dram_parameter("w3", [DC, P, 768], bf16, isOutput=False)
    wo = nc.declare_dram_parameter("wo", [H, P, D], bf16, isOutput=False)
    fc = nc.declare_dram_parameter("fc", [16, P, 64], bf16, isOutput=False)
    fs = nc.declare_dram_parameter("fs", [16, P, 64], bf16, isOutput=False)
    out = nc.declare_dram_parameter("out", [4, P, D], f32, isOutput=True)

    # one AllToAll per local head; block r = av^T for dest core r's 512 rows
    a2a_in = [nc.dram_tensor(f"a2a_in{i}", [NCORES, P, 512], bf16)
              for i in range(HL)]
    a2a_out = [nc.dram_tensor(f"a2a_out{i}", [NCORES, P, 512], bf16)
               for i in range(HL)]

    with TileContext(nc) as tc:
        with (
            tc.tile_pool(name="persist", bufs=1) as persist,
            tc.tile_pool(name="small", bufs=4) as small,
        ):
            # ---- persistent SBUF tensors --------------------------------
            # split per batch / per 8-tile group so Tile's per-tile dep
            # tracking doesn't serialize consumers on unrelated producers
            v_b = [persist.tile([P, 16, HL * HD], bf16, name=f"v_b{i}")
                   for i in range(B)]
            qk_b = [persist.tile([P, 4, 16, P], bf16, name=f"qk_b{i}")
                    for i in range(B)]
            dmask = persist.tile([P, P], f32)                # causal additive
            av_blk = [persist.tile([P, NCORES, 512], bf16, name=f"av_blk{i}")
                      for i in range(HL)]

            nc.gpsimd.memset(dmask, 0.0)
            nc.gpsimd.affine_select(
                out=dmask, in_=dmask, compare_op=OP.is_ge,
                fill=-1e9, base=0, pattern=[[-1, P]], channel_multiplier=1,
            )

            # ---- stage 1: QKV projection + LN + RoPE, groups of 8 -------
            with (
                tc.tile_pool(name="s1", bufs=1) as s1,
                tc.tile_pool(name="s1w", bufs=6) as s1w,
                tc.tile_pool(name="s1x", bufs=4) as s1x,
                tc.tile_pool(name="ps_qkv", bufs=3, space="PSUM") as ps_qkv,
            ):
                w3_sb = s1.tile([P, DC, 768], bf16)
                nc.sync.dma_start(w3_sb, w3[:, :, :].rearrange("a p c -> p a c"))
                fc_sb = s1.tile([P, 16, 64], bf16)
                fs_sb = s1.tile([P, 16, 64], bf16)
                nc.sync.dma_start(fc_sb, fc[:, :, :].rearrange("a p c -> p a c"))
                nc.sync.dma_start(fs_sb, fs[:, :, :].rearrange("a p c -> p a c"))
                for st in range(ST):
                    sg, r4 = st // 4, st % 4
                    if r4 == 0:
                        xt_g = s1x.tile([P, DC, 512], bf16, tag="xt",
                                        name=f"xt_{sg}")
                        nc.gpsimd.dma_start(
                            xt_g,
                            xt[:, :, ts(sg, 512)].rearrange("a p c -> p a c"))
                    ps = ps_qkv.tile([P, 768], f32, tag="psqkv",
                                     name=f"psqkv_{st}")
                    for dc in range(DC):
                        nc.tensor.matmul(
                            ps[:, 0:512], lhsT=xt_g[:, dc, ts(r4, P)],
                            rhs=w3_sb[:, dc, 0:512],
                            start=(dc == 0), stop=(dc == DC - 1))
                        nc.tensor.matmul(
                            ps[:, 512:768], lhsT=xt_g[:, dc, ts(r4, P)],
                            rhs=w3_sb[:, dc, 512:768],
                            start=(dc == 0), stop=(dc == DC - 1))
                    qkr = s1w.tile([P, 512], bf16, tag="qkr",
                                   name=f"qkr_{st}")
                    nc.scalar.copy(qkr, ps[:, 0:512])
                    nc.scalar.copy(v_b[st // 16][:, st % 16, :],
                                   ps[:, 512:768])
                    # LN stats (squares on DVE so ACT keeps one table set)
                    sq = s1w.tile([P, 512], bf16, tag="sq", name=f"sq_{st}")
                    nc.vector.tensor_tensor(sq, qkr, qkr, OP.mult)
                    st8 = s1w.tile([P, 8], f32, tag="st8", name=f"st8_{st}")
                    nc.vector.reduce_sum(
                        st8[:, 0:4],
                        ps[:, 0:512].rearrange("p (c x) -> p c x", c=4),
                        axis=mybir.AxisListType.X)
                    nc.vector.reduce_sum(
                        st8[:, 4:8],
                        sq.rearrange("p (c x) -> p c x", c=4),
                        axis=mybir.AxisListType.X)
                    mu4 = s1w.tile([P, 4], f32, tag="mu4", name=f"mu4_{st}")
                    var4 = s1w.tile([P, 4], f32, tag="var4", name=f"var4_{st}")
                    std4 = s1w.tile([P, 4], f32, tag="std4", name=f"std4_{st}")
                    rstd4 = s1w.tile([P, 4], f32, tag="rstd4",
                                     name=f"rstd4_{st}")
                    nmr4 = s1w.tile([P, 4], f32, tag="nmr4", name=f"nmr4_{st}")
                    nc.vector.tensor_scalar_mul(mu4, st8[:, 0:4], 1.0 / HD)
                    nc.vector.tensor_scalar(
                        var4, st8[:, 4:8], 1.0 / HD, EPS, OP.mult, OP.add)
                    nc.vector.tensor_tensor(std4, mu4, mu4, OP.mult)
                    nc.vector.tensor_tensor(var4, var4, std4, OP.subtract)
                    nc.scalar.sqrt(std4, var4)
                    nc.vector.reciprocal(rstd4, std4)
                    nc.vector.tensor_tensor(nmr4, mu4, rstd4, OP.mult)
                    nc.vector.tensor_scalar_mul(nmr4, nmr4, -1.0)
                    # LN apply + RoPE
                    sidx = st % 16
                    qkd = qk_b[st // 16]
                    for c4 in range(4):
                        nc.scalar.activation(
                            qkd[:, c4, sidx, :], qkr[:, ts(c4, P)],
                            AF.Identity,
                            bias=nmr4[:, c4:c4 + 1],
                            scale=rstd4[:, c4:c4 + 1])
                    cb = fc_sb[:, sidx:sidx + 1, :].to_broadcast((P, 4, 64))
                    sb_ = fs_sb[:, sidx:sidx + 1, :].to_broadcast((P, 4, 64))
                    e = qkd[:, :, sidx, 0:64]
                    o = qkd[:, :, sidx, 64:128]
                    t1 = s1w.tile([P, 4, 64], bf16, tag="t1")
                    t2 = s1w.tile([P, 4, 64], bf16, tag="t2")
                    t3 = s1w.tile([P, 4, 64], bf16, tag="t3")
                    t4 = s1w.tile([P, 4, 64], bf16, tag="t4")
                    nc.vector.tensor_tensor(t1, e, cb, OP.mult)
                    nc.vector.tensor_tensor(t2, o, sb_, OP.mult)
                    nc.vector.tensor_tensor(t3, e, sb_, OP.mult)
                    nc.vector.tensor_tensor(t4, o, cb, OP.mult)
                    nc.vector.tensor_tensor(e, t1, t2, OP.subtract)
                    nc.vector.tensor_tensor(o, t3, t4, OP.add)

            # ---- attention --------------------------------------------
            # software-pipelined per 4-row group: scores/exp/scale/transpose
            # for group g+1 are issued before PV of group g, so the PE has a
            # full group of runway while transposes complete.
            a2a_writes = [[], []]
            last_tr = [None]
            with (
                tc.tile_pool(name="att", bufs=2) as att,
                tc.tile_pool(name="attp", bufs=3) as attp,
                tc.tile_pool(name="attw", bufs=6) as attw,
                tc.tile_pool(name="ps_s", bufs=6, space="PSUM") as ps_s,
                tc.tile_pool(name="ps_av", bufs=2, space="PSUM") as ps_av,
            ):
                for hl in range(HL):
                    for b in range(B):
                        qt = att.tile([P, 16, P], bf16, tag="qt")
                        kt = att.tile([P, 16, P], bf16, tag="kt")
                        nc.sync.dma_start_transpose(
                            qt, qk_b[b][:, hl, :, :])
                        nc.sync.dma_start_transpose(
                            kt, qk_b[b][:, 2 + hl, :, :])

                        pts = {}

                        def scores_phase(g, qt=qt, kt=kt, pts=pts,
                                         hl=hl, b=b):
                            pt = attp.tile([P, 16, 512], bf16, tag="pt",
                                           name=f"pt_{hl}_{b}_{g}")
                            pts[g] = pt
                            for r in range(4):
                                qi = 4 * g + r
                                w_row = (qi + 1) * P
                                prow = attw.tile([P, S], bf16, tag="prow",
                                                 name=f"prow_{r}")
                                nfull = (qi // 4) * 4
                                for c0 in range(0, nfull, 4):
                                    pss = ps_s.tile([P, 512], f32, tag="pss")
                                    nc.tensor.matmul(
                                        pss, lhsT=qt[:, qi, :],
                                        rhs=kt[:, c0:c0 + 4, :],
                                        start=True, stop=True)
                                    nc.scalar.activation(
                                        prow[:, c0 * P:(c0 + 4) * P], pss,
                                        AF.Exp, scale=SCALE)
                                rw = (qi - nfull + 1) * P
                                pss = ps_s.tile([P, 512], f32, tag="pss")
                                nc.tensor.matmul(
                                    pss[:, 0:rw], lhsT=qt[:, qi, :],
                                    rhs=kt[:, nfull:qi + 1, :],
                                    start=True, stop=True)
                                nc.vector.tensor_tensor(
                                    pss[:, rw - P:rw], pss[:, rw - P:rw],
                                    dmask, OP.add)
                                nc.scalar.activation(
                                    prow[:, nfull * P:nfull * P + rw],
                                    pss[:, 0:rw], AF.Exp, scale=SCALE)
                                den = small.tile([P, 1], f32, tag="den")
                                rden = small.tile([P, 1], f32, tag="rden")
                                nc.vector.reduce_sum(
                                    den, prow[:, 0:w_row],
                                    axis=mybir.AxisListType.X)
                                nc.vector.reciprocal(rden, den)
                                prs = attw.tile([P, S], bf16, tag="prs",
                                                name=f"prs_{r}")
                                nc.vector.tensor_scalar_mul(
                                    prs[:, 0:w_row], prow[:, 0:w_row], rden)
                                last_tr[0] = nc.sync.dma_start_transpose(
                                    pt[:, 0:qi + 1, r * P:(r + 1) * P],
                                    prs[:, 0:w_row])

                        def pv_phase(g, pts=pts, hl=hl, b=b):
                            pt = pts.pop(g)
                            av = ps_av.tile([P, 512], f32, tag="av",
                                            name=f"av_{hl}_{b}_{g}")
                            for r in range(4):
                                qi = 4 * g + r
                                for kj in range(qi + 1):
                                    nc.tensor.matmul(
                                        av[:, r * P:(r + 1) * P],
                                        lhsT=v_b[b][:, kj,
                                                    hl * HD:(hl + 1) * HD],
                                        rhs=pt[:, kj, r * P:(r + 1) * P],
                                        start=(kj == 0), stop=(kj == qi))
                                nc.vector.tensor_copy(
                                    av_blk[hl][:, 4 * b + g,
                                               r * P:(r + 1) * P],
                                    av[:, r * P:(r + 1) * P])

                        for g in range(6):
                            if g < 4:
                                scores_phase(g)
                            if g >= 2:
                                pv_phase(g - 2)

                    for j in range(NCORES):
                        wr = nc.gpsimd.dma_start(a2a_in[hl][j],
                                                 av_blk[hl][:, j, :])
                        a2a_writes[hl].append(wr)
            # ---- AllToAll x2 (second overlaps even-head wo matmuls) ----
            colls = []
            for hl in range(HL):
                coll = nc.gpsimd.collective_compute(
                    "AllToAll", OP.bypass,
                    ins=[a2a_in[hl][:, :, :]],
                    outs=[a2a_out[hl][:, :, :]],
                    replica_groups=[list(range(NCORES))],
                )
                for wr in a2a_writes[hl]:
                    add_dep_helper(coll.ins, wr.ins,
                                   reason="a2a needs staged inputs")
                add_dep_helper(coll.ins, last_tr[0].ins, sync=False,
                               reason="keep collective after all transposes")
                colls.append(coll)

            # ---- wo: out rows = this core's 512, all 16 heads ----------
            # head h lives in a2a_out[h % 2], block h // 2; process heads of
            # a2a#0 first so they overlap a2a#1's wire time.
            with (
                tc.tile_pool(name="wop", bufs=1) as wop,
                tc.tile_pool(name="wopf", bufs=10) as wopf,
                tc.tile_pool(name="wow", bufs=3) as wow,
                tc.tile_pool(name="ps_o", bufs=2, space="PSUM") as ps_o,
            ):
                avr = [wop.tile([P, NCORES, 512], bf16, name=f"avr{i}")
                       for i in range(HL)]
                for i in range(HL):
                    for src in range(NCORES):
                        rd = nc.scalar.dma_start(avr[i][:, src, :],
                                                 a2a_out[i][src])
                        add_dep_helper(rd.ins, colls[i].ins,
                                       reason="read a2a output")
                horder = [2 * c for c in range(NCORES)] + \
                         [2 * c + 1 for c in range(NCORES)]
                for nchk in range(4):
                    pss4 = [ps_o.tile([P, 512], f32, tag=f"po{st4}",
                                      name=f"po_{nchk}_{st4}")
                            for st4 in range(4)]
                    for ih, h in enumerate(horder):
                        wot = wopf.tile([P, 512], bf16, tag="wot",
                                        name=f"wot_{nchk}_{h}")
                        nc.gpsimd.dma_start(wot, wo[h, :, ts(nchk, 512)])
                        for st4 in range(4):
                            nc.tensor.matmul(
                                pss4[st4],
                                lhsT=avr[h % 2][:, h // 2, ts(st4, P)],
                                rhs=wot, start=(ih == 0), stop=(ih == H - 1))
                    for st4 in range(4):
                        ot = wow.tile([P, 512], f32, tag="ot")
                        nc.vector.tensor_copy(ot, pss4[st4])
                        nc.sync.dma_start(out[st4, :, ts(nchk, 512)], ot)

    nc.compile()
    _NC_CACHE["nc"] = nc
    return nc


def _perm_even_odd():
    # column permutation de-interleaving RoPE pairs: [0,2,...,126,1,3,...,127]
    return np.concatenate([np.arange(0, HD, 2), np.arange(1, HD, 2)])


def prep_in_maps(inputs):
    import ml_dtypes

    bf = ml_dtypes.bfloat16
    x = np.asarray(inputs["x"], dtype=np.float32).reshape(BS, D)
    wq = np.asarray(inputs["wq"], dtype=np.float32)
    wk = np.asarray(inputs["wk"], dtype=np.float32)
    wv = np.asarray(inputs["wv"], dtype=np.float32)
    wo = np.asarray(inputs["wo"], dtype=np.float32)
    fc = np.asarray(inputs["freqs_cos"], dtype=np.float32)
    fs = np.asarray(inputs["freqs_sin"], dtype=np.float32)

    xt = np.ascontiguousarray(x.T).astype(bf).reshape(DC, P, BS)
    wo_b = np.ascontiguousarray(wo).astype(bf).reshape(H, P, D)
    fc_b = np.ascontiguousarray(fc.reshape(16, P, 64)).astype(bf)
    fs_b = np.ascontiguousarray(fs.reshape(16, P, 64)).astype(bf)

    perm = _perm_even_odd()
    in_maps = []
    for c in range(NCORES):
        cols = []
        for w in (wq, wk):
            for hl in range(HL):
                h = HL * c + hl
                cols.append(w[:, h * HD:(h + 1) * HD][:, perm])
        cols.append(wv[:, HL * c * HD:(HL * c + HL) * HD])
        w3_c = np.concatenate(cols, axis=1).astype(bf).reshape(DC, P, 768)
        in_maps.append({
            "xt": xt, "w3": np.ascontiguousarray(w3_c), "wo": wo_b,
            "fc": fc_b, "fs": fs_b,
        })
    return in_maps


def assemble(results):
    parts = [np.asarray(r["out"], dtype=np.float32).reshape(512, D)
             for r in results]
    return np.concatenate(parts, axis=0).reshape(B, S, D)


def _is_supported(inputs):
    mask = np.asarray(inputs["mask"], dtype=np.float32)
    if mask.shape != (S, S):
        return False
    tril = np.tril(np.ones((S, S), dtype=bool))
    if not (np.all(mask[tril] == 0.0) and np.all(mask[~tril] <= -1e8)):
        return False
    for k, v in (("q_ln_w", 1.0), ("q_ln_b", 0.0),
                 ("k_ln_w", 1.0), ("k_ln_b", 0.0)):
        if not np.allclose(np.asarray(inputs[k], dtype=np.float32), v):
            return False
    return True


def _kernel_numpy(inputs):
    x = np.asarray(inputs["x"], dtype=np.float32)
    fc, fs = np.asarray(inputs["freqs_cos"]), np.asarray(inputs["freqs_sin"])
    mask = np.asarray(inputs["mask"])
    wq, wk, wv, wo = (np.asarray(inputs[k]) for k in ("wq", "wk", "wv", "wo"))
    qw, qb = np.asarray(inputs["q_ln_w"]), np.asarray(inputs["q_ln_b"])
    kw, kb = np.asarray(inputs["k_ln_w"]), np.asarray(inputs["k_ln_b"])

    def ln(t, w, b):
        mu = t.mean(-1, keepdims=True)
        var = ((t - mu) ** 2).mean(-1, keepdims=True)
        return (t - mu) / np.sqrt(var + EPS) * w + b

    def rope(t):
        e, o = t[..., 0::2], t[..., 1::2]
        c = fc[None, :, None, :]
        s = fs[None, :, None, :]
        outp = np.empty_like(t)
        outp[..., 0::2] = e * c - o * s
        outp[..., 1::2] = e * s + o * c
        return outp

    b, s, _ = x.shape
    q = rope(ln((x @ wq).reshape(b, s, H, HD), qw, qb))
    k = rope(ln((x @ wk).reshape(b, s, H, HD), kw, kb))
    v = (x @ wv).reshape(b, s, H, HD)
    outp = np.empty((b, s, H, HD), dtype=np.float32)
    for bi in range(b):
        for h in range(H):
            sc = (q[bi, :, h, :] @ k[bi, :, h, :].T) * SCALE + mask
            sc -= sc.max(-1, keepdims=True)
            p = np.exp(sc)
            p /= p.sum(-1, keepdims=True)
            outp[bi, :, h, :] = p @ v[bi, :, h, :]
    return (outp.reshape(b, s, D) @ wo).astype(np.float32)


def kernel(**inputs) -> np.ndarray:
    if not _is_supported(inputs):
        return _kernel_numpy(inputs)
    from concourse.bass_utils import run_bass_kernel_spmd

    nc = build_nc()
    in_maps = prep_in_maps(inputs)
    res = run_bass_kernel_spmd(nc, in_maps, list(range(NCORES)))
    return assemble(res.results)


if __name__ == "__main__":
    import reference

    inputs = {k: np.asarray(v) for k, v in reference.setup_inputs().items()}
    expected = np.asarray(reference.reference(**inputs))
    actual = kernel(**inputs)
    rel = np.abs(actual - expected).max() / (np.abs(expected).max() + 1e-30)
    print(f"Relative error: {rel:.3e}")
